# revision 2
# baseline (speedup 1.0000x reference)
"""Trainium2 Bass kernel for nn_LlamaAttention_cam (sparse attention + CaM merge).

Sharding: tensor-parallel over heads across 8 NeuronCores (2 heads/core).
Each core computes its heads' QKV projections, RoPE, masked attention
(start+recent keep mask), CaM rank-1 correction for the last chunk, and a
partial o_proj (its 256 columns of x against the matching 256 rows of Wo^T).
The host sums the 8 partial outputs (the head-parallel o_proj reduction).

Performance structure on top of the fp16 pipeline version:
- All four projections (Q/K/V and o_proj) run as fp8e4 DoubleRow matmuls
  with a 3-pass hi+lo decomposition: operands are pre-split into e4m3
  hi/lo pairs (lo = residual of hi at the same PSUM scale, pre-scaled so
  lo stays above the e4m3 subnormal floor). Each DoubleRow instruction
  contracts K=256 at 0.5 cycles/output-row, so the 3 passes cost 0.75x
  the fp16 cycles while keeping ~fp16 accuracy (hi+lo ~ 8 mantissa bits).
- The attention core (scores, exp, AV) stays fp16: its contractions are
  K=128/short and plain-fp8 would add ~2% noise vs the 2e-2 gate.
- Scales are folded for free: trig tables absorb the Q/K PSUM scales,
  the V evacuation absorbs the V scale and the x32 o_proj input scale,
  and the host divides the summed partial outputs by the o_proj PSUM
  scale (po is stored x4096).
- Only the 1023 kept keys (start 0:204 + recent 1229:2048, padded to 1024)
  are projected/roped for K and V.
- Inputs are host-pre-tiled so each tensor loads with O(1) large DMAs;
  weights are SBUF-resident. hi-parts load before lo-parts and the two
  hi-passes of each chain are emitted first, so compute starts earlier.
"""

import sys

for _p in ("/opt/trn_rl_repo",):
    if _p not in sys.path:
        sys.path.append(_p)

import numpy as np
import ml_dtypes

import concourse.bass as bass
import concourse.mybir as mybir
import concourse.tile as tile
from concourse import bacc, bass_isa, bass_utils

F32 = mybir.dt.float32
F16 = mybir.dt.float16
F8 = mybir.dt.float8e4
E4 = ml_dtypes.float8_e4m3
DR = mybir.MatmulPerfMode.DoubleRow
AF = mybir.ActivationFunctionType

T = 2048
DM = 2048
H = 16
D = 128
NCORES = 8
HL = H // NCORES          # heads per core = 2
JC = HL * D               # local width = 256
SB = 204                  # start keep
RB = 819                  # recent keep
EV = T - RB               # 1229 (first recent key; CaM source row)
NK = 1024                 # gathered keys = 1023 kept + 1 pad
KC = DM // 128            # 16 model-dim chunks
NP = KC // 2              # 8 kc slot-pairs for DoubleRow
TB = T // 512             # 4 t-blocks of 512
TI = T // 128             # 16 t-chunks of 128
NB = NK // 128            # 8 gathered key blocks
GEV_P = 76                # key EV lives at gathered block 1, partition 76

# fp8 pre-scales (powers of 2; chosen so each stored tensor has std ~2 and
# the lo-residuals clear the e4m3 subnormal floor at the shared PSUM scale)
S_I = 2.0                 # hidden-state scale
S_WQ = 1024.0             # Wq scale (also carries 1/sqrt(D))
S_WK = 128.0
S_WV = 128.0
S_WO = 128.0
S_X = 32.0                # attention-output scale into o_proj
PS_Q = S_I * S_WQ         # 2048: folded out via trigF tables
PS_K = S_I * S_WK         # 256: folded out via trigK tables
V_EVAC = S_X / (S_I * S_WV)   # 1/8: psumV -> vt (= v * 32)
PS_O = S_X * S_WO         # 4096: folded out on the host after the core sum


def _build_nc():
    nc = bacc.Bacc("TRN2", target_bir_lowering=False, debug=False,
                   num_devices=NCORES)
    # host-pre-tiled inputs (see make_in_maps for layouts)
    hst_hi = nc.dram_tensor("hst_hi", [128, TB, KC, 512], F8,
                            kind="ExternalInput").ap()
    hst_lo = nc.dram_tensor("hst_lo", [128, TB, KC, 512], F8,
                            kind="ExternalInput").ap()
    hsk_hi = nc.dram_tensor("hsk_hi", [128, KC, NK], F8,
                            kind="ExternalInput").ap()
    hsk_lo = nc.dram_tensor("hsk_lo", [128, KC, NK], F8,
                            kind="ExternalInput").ap()
    wq_hi = nc.dram_tensor("wq_hi", [128, KC, JC], F8,
                           kind="ExternalInput").ap()
    wq_lo = nc.dram_tensor("wq_lo", [128, KC, JC], F8,
                           kind="ExternalInput").ap()
    wk_hi = nc.dram_tensor("wk_hi", [128, KC, JC], F8,
                           kind="ExternalInput").ap()
    wk_lo = nc.dram_tensor("wk_lo", [128, KC, JC], F8,
                           kind="ExternalInput").ap()
    wv_hi = nc.dram_tensor("wv_hi", [128, KC, JC], F8,
                           kind="ExternalInput").ap()
    wv_lo = nc.dram_tensor("wv_lo", [128, KC, JC], F8,
                           kind="ExternalInput").ap()
    wo_hi = nc.dram_tensor("wo_hi", [128, HL, DM], F8,
                           kind="ExternalInput").ap()
    wo_lo = nc.dram_tensor("wo_lo", [128, HL, DM], F8,
                           kind="ExternalInput").ap()
    trig = nc.dram_tensor("trig", [128, 2 * T + 2 * NK], F16,
                          kind="ExternalInput").ap()
    misc = nc.dram_tensor("misc", [128, 5], F32, kind="ExternalInput").ap()
    po = nc.dram_tensor("po", [T, DM], F16, kind="ExternalOutput").ap()

    with tile.TileContext(nc) as tc:
        with (
            tc.tile_pool(name="res", bufs=1) as pres,
            tc.tile_pool(name="hst", bufs=2) as phst,
            tc.tile_pool(name="raw", bufs=4) as praw,
            tc.tile_pool(name="e16", bufs=26) as pe16,
            tc.tile_pool(name="dn16", bufs=5) as pdn,
            tc.tile_pool(name="osb", bufs=3) as posb,
            tc.tile_pool(name="rbf", bufs=4) as prbf,
            tc.tile_pool(name="row", bufs=4) as prow,
            tc.tile_pool(name="ps", bufs=4, space="PSUM") as pps,
            tc.tile_pool(name="av", bufs=3, space="PSUM") as pav,
            tc.tile_pool(name="psd", bufs=1, space="PSUM") as ppsd,
        ):
            # ---- resident loads on the SP queue, interleaved hi/lo in
            # 2-kc chunks so each K-proj pass's operands land just in time;
            # wv before the V phase, then the tb3+tb0 hst pairs, wq, so the
            # whole front of the pipeline is never DMA-starved ----
            # Front loads stream on FOUR parallel queues so the K/V/Q(tb3)
            # phase is PE-bound, not DMA-bound:
            #   SP:   wkh head, kh chunks, wvh, hst3-hi, wqh  (~13us)
            #   ACT:  wkh tail, wkl, kl chunks 0-3, wvl       (~8us, then exp)
            #   Pool: kl chunks 4-7, hst3-lo, wql             (~8us)
            #   DVE:  trig, misc                              (~5us)
            wkh = pres.tile([128, KC, JC], F8, tag="wkh")
            nc.sync.dma_start(wkh[:, 0:2, :], wk_hi[:, 0:2, :])
            kh = pres.tile([128, KC, NK], F8, tag="kh")
            kl = pres.tile([128, KC, NK], F8, tag="kl")
            wkl = pres.tile([128, KC, JC], F8, tag="wkl")
            trig_sb = pres.tile([128, 2 * T + 2 * NK], F16, tag="trig")
            cosF = trig_sb[:, 0:T]
            sinF = trig_sb[:, T:2 * T]
            cosK = trig_sb[:, 2 * T:2 * T + NK]
            sinK = trig_sb[:, 2 * T + NK:2 * T + 2 * NK]
            # kh then kl round-robin across all three DMA queues, so all
            # K inputs land by ~6us and the K phase tracks the stream:
            #   SP:  wkh head, kh c0/3/6, kl c1/4/7, wvh, hst3-hi, wqh
            #   ACT: wkh tail, kh c1/4/7, kl c2/5, wvl
            #   Pool: wkl, kh c2/5, kl c0/3/6, K trig, hst3-lo, wql
            nc.scalar.dma_start(wkh[:, 2:, :], wk_hi[:, 2:, :])
            nc.gpsimd.dma_start(wkl[:], wk_lo[:])
            _Q3 = (nc.sync, nc.scalar, nc.gpsimd)
            for c in range(8):
                _Q3[c % 3].dma_start(kh[:, 2 * c:2 * c + 2, :],
                                     hsk_hi[:, 2 * c:2 * c + 2, :])
            for c in range(8):
                _Q3[(c + 2) % 3].dma_start(kl[:, 2 * c:2 * c + 2, :],
                                           hsk_lo[:, 2 * c:2 * c + 2, :])
            nc.gpsimd.dma_start(trig_sb[:, 2 * T:], trig[:, 2 * T:])
            wvh = pres.tile([128, KC, JC], F8, tag="wvh")
            nc.sync.dma_start(wvh[:], wv_hi[:])
            wvl = pres.tile([128, KC, JC], F8, tag="wvl")
            nc.scalar.dma_start(wvl[:], wv_lo[:])
            wqh = pres.tile([128, KC, JC], F8, tag="wqh")
            wql = pres.tile([128, KC, JC], F8, tag="wql")
            misc_sb = pres.tile([128, 5], F32, tag="misc")
            # wo reuses the hsk slots: kh/kl die after the K/V phase
            # (~25us) and wo is first read by oproj (~45us)
            woh = pres.tile([128, HL, DM], F8, tag="kh", name="woh")
            wol = pres.tile([128, HL, DM], F8, tag="kl", name="wol")

            ones16 = pres.tile([128, 1], F16, tag="ones16")
            mask16 = pres.tile([128, 1], F16, tag="mask16")

            qrT = [pres.tile([128, T], F16, tag=f"qrT{l}", name=f"qrT{l}")
                   for l in range(HL)]
            krT = [pres.tile([128, NK], F16, tag=f"krT{l}", name=f"krT{l}")
                   for l in range(HL)]
            vt = pres.tile([128, NB * JC], F16, tag="vt")
            outT = [pres.tile([128, T], F16, tag=f"outT{l}", name=f"outT{l}")
                    for l in range(HL)]
            xhi = pres.tile([128, HL, T], F8, tag="xhi")
            xlo = pres.tile([128, HL, T], F8, tag="xlo")

            def mm3(ps, wh, wl, mh, ml, first, last):
                """3-pass hi/lo DoubleRow accumulation into ps.

                Emits hi*hi, lo*hi, hi*lo for one K=256 slot-pair; pass
                order per chain is handled by the caller (hi passes of all
                pairs first so compute can start before lo tensors land).
                """
                nc.tensor.matmul(ps, wh, mh, start=first, stop=False,
                                 perf_mode=DR)
                nc.tensor.matmul(ps, wl, mh, start=False, stop=False,
                                 perf_mode=DR)
                nc.tensor.matmul(ps, wh, ml, start=False, stop=last,
                                 perf_mode=DR)

            def proj_chain(ps_half, w_hi, w_lo, w_sl, m_hi, m_lo, m_sl):
                """One 256-wide output half: K=2048 as 8 slot-pairs x 3
                passes, hi passes first."""
                for c in range(NP):
                    nc.tensor.matmul(
                        ps_half, w_hi[:, 2 * c:2 * c + 2, w_sl],
                        m_hi[:, 2 * c:2 * c + 2, m_sl],
                        start=(c == 0), stop=False, perf_mode=DR)
                for c in range(NP):
                    nc.tensor.matmul(
                        ps_half, w_lo[:, 2 * c:2 * c + 2, w_sl],
                        m_hi[:, 2 * c:2 * c + 2, m_sl],
                        start=False, stop=False, perf_mode=DR)
                for c in range(NP):
                    nc.tensor.matmul(
                        ps_half, w_hi[:, 2 * c:2 * c + 2, w_sl],
                        m_lo[:, 2 * c:2 * c + 2, m_sl],
                        start=False, stop=(c == NP - 1), perf_mode=DR)

            # ---------------- K projection + rope (gathered keys) ----------
            def rope(ps, dstT, cos_sl, sin_sl, fast=False):
                raw = praw.tile([128, 512], F16, tag="raw")
                # gpsimd cannot read PSUM on HW: evac via ACT (early, idle)
                # or DVE
                if fast:
                    nc.scalar.copy(raw[:], ps[:])
                else:
                    nc.vector.tensor_copy(raw[:], ps[:])
                sh = praw.tile([128, 512], F16, tag="raw")
                # SWDGE path keeps these shuffles out of the SP HWDGE FIFO
                # (which carries the big streaming loads); the ACT HWDGE ring
                # serves the schedule-critical first Q t-block faster.
                dma = nc.scalar.dma_start if fast else nc.gpsimd.dma_start
                dma(sh[0:64, :], raw[64:128, :])
                dma(sh[64:128, :], raw[0:64, :])
                nc.vector.tensor_mul(raw[:], raw[:], cos_sl)
                nc.vector.tensor_mul(sh[:], sh[:], sin_sl)
                nc.vector.tensor_add(dstT, raw[:], sh[:])

            def rope2(psA, psB, dstT, cos_sl, sin_sl):
                # like rope() but over a pair of 256-wide psum tiles
                raw = praw.tile([128, 512], F16, tag="raw")
                nc.scalar.copy(raw[:, 0:256], psA[:])
                nc.scalar.copy(raw[:, 256:512], psB[:])
                sh = praw.tile([128, 512], F16, tag="raw")
                nc.gpsimd.dma_start(sh[0:64, :], raw[64:128, :])
                nc.gpsimd.dma_start(sh[64:128, :], raw[0:64, :])
                nc.vector.tensor_mul(raw[:], raw[:], cos_sl)
                nc.vector.tensor_mul(sh[:], sh[:], sin_sl)
                nc.vector.tensor_add(dstT, raw[:], sh[:])

            # K projection: all 8 [128,256] chains at once (using every
            # PSUM bank: 4 from ps, 3 from av, 1 from dn), chunk-major in
            # DMA-arrival order — hi passes track the kh stream, then lo
            # passes run at full speed once kl has landed.
            kunits = [(l, kb, h) for l in range(HL) for kb in range(2)
                      for h in range(2)]
            psk = []
            for i in range(8):
                pool, tg = ((pps, "ps") if i < 4 else
                            (pav, "av") if i < 7 else (ppsd, "dn"))
                psk.append(pool.tile([128, 256], F32, tag=tg,
                                     name=f"psk{i}"))

            def kslice(kb, h):
                return slice(kb * 512 + h * 256, kb * 512 + h * 256 + 256)

            HI_ORDER = (0, 3, 1, 2, 6, 4, 5, 7)
            LO_ORDER = (1, 0, 4, 2, 3, 7, 5, 6)
            for ci, c in enumerate(HI_ORDER):
                for u, (l, kb, h) in enumerate(kunits):
                    nc.tensor.matmul(
                        psk[u][:], wkh[:, 2 * c:2 * c + 2, l * D:l * D + D],
                        kh[:, 2 * c:2 * c + 2, kslice(kb, h)],
                        start=(ci == 0), stop=False, perf_mode=DR)
            for ci, c in enumerate(LO_ORDER):
                last = ci == NP - 1
                for u, (l, kb, h) in enumerate(kunits):
                    nc.tensor.matmul(
                        psk[u][:], wkl[:, 2 * c:2 * c + 2, l * D:l * D + D],
                        kh[:, 2 * c:2 * c + 2, kslice(kb, h)],
                        start=False, stop=False, perf_mode=DR)
                for u, (l, kb, h) in enumerate(kunits):
                    nc.tensor.matmul(
                        psk[u][:], wkh[:, 2 * c:2 * c + 2, l * D:l * D + D],
                        kl[:, 2 * c:2 * c + 2, kslice(kb, h)],
                        start=False, stop=last, perf_mode=DR)
            for l in range(HL):
                for kb in range(2):
                    k5 = slice(kb * 512, kb * 512 + 512)
                    u0 = kunits.index((l, kb, 0))
                    rope2(psk[u0], psk[u0 + 1], krT[l][:, k5],
                          cosK[:, k5], sinK[:, k5])

            # ---------------- V projection (gathered keys) ------------------
            def vproj(kb):
                psv = pps.tile([128, JC], F32, tag="ps")
                ksl = slice(kb * 128, kb * 128 + 128)
                for c in range(NP):
                    nc.tensor.matmul(psv[:], kh[:, 2 * c:2 * c + 2, ksl],
                                     wvh[:, 2 * c:2 * c + 2, :],
                                     start=(c == 0), stop=False, perf_mode=DR)
                for c in range(NP):
                    nc.tensor.matmul(psv[:], kl[:, 2 * c:2 * c + 2, ksl],
                                     wvh[:, 2 * c:2 * c + 2, :],
                                     start=False, stop=False, perf_mode=DR)
                for c in range(NP):
                    nc.tensor.matmul(psv[:], kh[:, 2 * c:2 * c + 2, ksl],
                                     wvl[:, 2 * c:2 * c + 2, :],
                                     start=False, stop=(c == NP - 1),
                                     perf_mode=DR)
                # vt = v * S_X (32): x32 comes out of the AV matmul for free
                nc.vector.tensor_scalar_mul(vt[:, kb * JC:(kb + 1) * JC],
                                            psv[:], V_EVAC)

            def hst_load(tb):
                hsth = phst.tile([128, KC, 512], F8, tag="hsth")
                hstl = phst.tile([128, KC, 512], F8, tag="hstl")
                nc.sync.dma_start(hsth[:], hst_hi[:, tb])
                nc.sync.dma_start(hstl[:], hst_lo[:, tb])
                return hsth, hstl

            # preload tb3 + tb0 hst pairs and wq behind the K/V streams;
            # later tb loads are issued from inside the Q loop
            hst_tiles = {}
            hsth3 = phst.tile([128, KC, 512], F8, tag="hsth")
            hstl3 = phst.tile([128, KC, 512], F8, tag="hstl")
            nc.sync.dma_start(hsth3[:], hst_hi[:, 3])
            nc.gpsimd.dma_start(hstl3[:], hst_lo[:, 3])
            hst_tiles[3] = (hsth3, hstl3)
            nc.sync.dma_start(wqh[:], wq_hi[:])
            nc.gpsimd.dma_start(wql[:], wq_lo[:])
            nc.gpsimd.dma_start(trig_sb[:, 0:T], trig[:, 0:T])
            nc.gpsimd.dma_start(trig_sb[:, T:2 * T], trig[:, T:2 * T])
            nc.gpsimd.dma_start(misc_sb[:], misc[:])
            nc.vector.tensor_copy(ones16[:], misc_sb[:, 0:1])
            nc.vector.tensor_copy(mask16[:], misc_sb[:, 1:2])
            hst_tiles[0] = hst_load(0)

            # blocks 6-7 are emitted after Q(tb3) so the PE has work while
            # the tb3 rope chain (copy -> swap DMA -> muls) completes
            for kb in range(NB - 2):
                vproj(kb)

            # ---------------- fused Q-proj / attention / o_proj pipeline ----
            def qproj(qi, tb):
                ts5 = slice(tb * 512, tb * 512 + 512)
                hsth, hstl = hst_tiles.pop(tb)
                if qi == 0:
                    hst_tiles[1] = hst_load(1)
                elif qi == 1:
                    nc.sync.dma_start(woh[:], wo_hi[:])
                    hst_tiles[2] = hst_load(2)
                elif qi == 2:
                    nc.sync.dma_start(wol[:], wo_lo[:])
                for l in range(HL):
                    ps = pps.tile([128, 512], F32, tag="ps")
                    for h in range(2):
                        proj_chain(ps[:, h * 256:h * 256 + 256],
                                   wqh, wql, slice(l * D, l * D + D),
                                   hsth, hstl,
                                   slice(h * 256, h * 256 + 256))
                    rope(ps, qrT[l][:, ts5], cosF[:, ts5], sinF[:, ts5],
                         fast=(qi == 0))

            def scores_block(tb, l, kb):
                ts5 = slice(tb * 512, tb * 512 + 512)
                ps = pps.tile([128, 512], F32, tag="ps")
                nc.tensor.matmul(ps[:], krT[l][:, kb * 128:(kb + 1) * 128],
                                 qrT[l][:, ts5], start=True, stop=True)
                e = pe16.tile([128, 512], F16, tag="e")
                nc.scalar.activation(e[:], ps[:], AF.Exp)
                return e

            def oproj_ti(ti, evac=("dve", "act"), split_store=False):
                # x (hi+lo, x32) against wo (hi+lo, x128): po holds 4096x
                # the partial o_proj; the host divides after the core-sum.
                # Two 256-col chains share one 512-wide psum (sequential
                # chains; zero-region safe) so each evac is a single wide
                # copy, rotated across engines to keep pace with the PE.
                osb = posb.tile([128, DM], F16, tag="osb")
                tsl = slice(ti * 128, ti * 128 + 128)
                ev = {"dve": nc.vector.tensor_copy,
                      "act": nc.scalar.copy}
                for mb in range(DM // 512):
                    pso = pps.tile([128, 512], F32, tag="ps")
                    for half in range(2):
                        cb = 2 * mb + half
                        csl = slice(cb * 256, cb * 256 + 256)
                        mm3(pso[:, half * 256:half * 256 + 256],
                            xhi[:, :, tsl], xlo[:, :, tsl],
                            woh[:, :, csl], wol[:, :, csl], True, True)
                    msl = slice(mb * 512, (mb + 1) * 512)
                    ev[evac[mb % len(evac)]](osb[:, msl], pso[:])
                    # piece-store as soon as each 512-col evac lands; late
                    # tiles alternate SP/ACT so the drain isn't serialized
                    sq = nc.scalar if (split_store and mb % 2 == 1) \
                        else nc.sync
                    sq.dma_start(po[tsl, msl], osb[:, msl])

            def xsplit(l, c0, c1):
                # outT holds x*32 (f16); split to e4m3 hi + lo residual
                sl = slice(c0, c1)
                nc.gpsimd.tensor_copy(xhi[:, l, sl], outT[l][:, sl])
                nc.vector.tensor_sub(xlo[:, l, sl], outT[l][:, sl],
                                     xhi[:, l, sl])

            # Pipeline state: "fin" is the previous unit's denominator tail
            # (all-reduce + reciprocal + normalize), emitted mid-way through
            # the NEXT unit's AV loop so its cross-engine latency never
            # head-of-line-blocks an engine FIFO. "cams" are deferred CaM
            # scalar chains (one per tb-3 head), emitted two units later.
            state = {"Eq": [], "fin": None, "cams": [], "cam2": []}

            def make_cam(l, E0, E1, erow16, vrow, dnrow, rrow):
                def cam_a():
                    # sums over gathered keys 0..204 for q in [1792,2048)
                    psst = ppsd.tile([1, 256], F32, tag="dn")
                    nc.tensor.matmul(psst[:], ones16[:], E0[:, 256:512],
                                     start=True, stop=False)
                    nc.tensor.matmul(psst[:], mask16[:], E1[:, 256:512],
                                     start=False, stop=True)
                    erow = prow.tile([1, 256], F32, tag="row256")
                    nc.vector.tensor_copy(erow[:], erow16[:])
                    srec = prow.tile([1, 256], F32, tag="row256")
                    nc.vector.tensor_sub(srec[:], dnrow[0:1, 256:512],
                                         psst[:])
                    nc.vector.tensor_sub(srec[:], srec[:], erow[:])
                    # rrow is 1/denom (unscaled); p = num/mean is invariant
                    # to any common scale on rrow.
                    r_last = rrow[0:1, 511:512]
                    num = prow.tile([1, 1], F32, tag="sc")
                    nc.vector.tensor_mul(num[:], erow[0:1, 255:256], r_last)
                    mean = prow.tile([1, 1], F32, tag="sc")
                    nc.vector.tensor_mul(mean[:], srec[0:1, 255:256], r_last)
                    nc.vector.tensor_scalar_mul(mean[:], mean[:],
                                                1.0 / (RB - 1.0))
                    nc.vector.tensor_scalar_add(mean[:], mean[:], 1e-6)
                    um = prow.tile([1, 1], F32, tag="sc")
                    nc.vector.tensor_mul(um[:], misc_sb[0:1, 3 + l:4 + l],
                                         mean[:])
                    bern = prow.tile([1, 1], F32, tag="sc")
                    nc.vector.tensor_tensor(bern[:], um[:], num[:],
                                            mybir.AluOpType.is_lt)
                    bs = prow.tile([1, 1], F32, tag="sc")
                    nc.vector.tensor_scalar_mul(bs[:], bern[:], 1.0 / RB)
                    coef = prow.tile([1, 256], F16, tag="coefh")
                    nc.vector.tensor_scalar_mul(coef[:], srec[:], bs[:])
                    state["cam2"].append(make_cam_b(l, coef, vrow, rrow))
                return cam_a

            def make_cam_b(l, coef, vrow, rrow):
                def cam_b():
                    # vrow is v*32, so corr lands already in outT's x32 scale
                    pscr = pps.tile([128, 256], F32, tag="ps")
                    nc.tensor.matmul(pscr[:], vrow[:], coef[:],
                                     start=True, stop=True)
                    rbb = prbf.tile([128, 512], F32, tag="rbf")
                    nc.gpsimd.partition_broadcast(rbb[:, 0:256],
                                                  rrow[0:1, 256:512])
                    corr = pdn.tile([128, 512], F16, tag="dn16")
                    nc.vector.tensor_mul(corr[:, 0:256], pscr[:],
                                         rbb[:, 0:256])
                    nc.vector.tensor_add(outT[l][:, 1792:2048],
                                         outT[l][:, 1792:2048],
                                         corr[:, 0:256])
                    xsplit(l, 1792, 2048)
                return cam_b

            def av_unit(tb, l, nxt):
                # E sets are produced two units ahead (state["Eq"] holds the
                # pending sets) so the ACT exp stream never paces the AV
                # chain; nxt here is the unit TWO ahead of this one.
                E_cur = state["Eq"].pop(0)
                ready_a = state["cams"]
                state["cams"] = []
                ready_b = state["cam2"]
                state["cam2"] = []
                ts5 = slice(tb * 512, tb * 512 + 512)
                if tb == 3:
                    # hoist the CaM row extractions: by the time the deferred
                    # chain runs, these SWDGE DMAs are long done
                    erow16 = prow.tile([1, 256], F16, tag="row256h")
                    nc.gpsimd.dma_start(erow16[:],
                                        E_cur[1][GEV_P:GEV_P + 1, 256:512])
                    vrow = prow.tile([1, D], F16, tag="vrowh")
                    nc.gpsimd.dma_start(
                        vrow[:],
                        vt[GEV_P:GEV_P + 1, JC + l * D:JC + l * D + D])
                psav = pav.tile([128, 512], F32, tag="av")
                E_nxt = []
                # softmax denominator off the PE: pairwise-sum the E blocks
                # on the DVE (interleaved with the AV chain), then one
                # gpsimd partition all-reduce.
                tsum = [None, None]
                for kb in range(NB):
                    nc.tensor.matmul(
                        psav[:], vt[:, kb * JC + l * D:kb * JC + l * D + D],
                        E_cur[kb][:], start=(kb == 0), stop=(kb == NB - 1))
                    if kb == 3 and state["fin"] is not None:
                        state["fin"]()
                        state["fin"] = None
                    if kb == 5 and ready_a:
                        ready_a.pop(0)()
                    if kb == 6 and ready_b:
                        ready_b.pop(0)()
                    if kb % 2 == 1:
                        half = kb // 4
                        tp = pdn.tile([128, 512], F16, tag="dn16")
                        nc.gpsimd.tensor_add(tp[:], E_cur[kb - 1][:],
                                             E_cur[kb][:])
                        if tsum[half] is None:
                            tsum[half] = tp
                        else:
                            nc.vector.tensor_add(tsum[half][:],
                                                 tsum[half][:], tp[:])
                    if nxt is not None:
                        E_nxt.append(scores_block(nxt[0], nxt[1], kb))
                if nxt is not None:
                    state["Eq"].append(E_nxt)
                state["cams"] = ready_a + state["cams"]
                state["cam2"] = ready_b + state["cam2"]
                nc.vector.tensor_add(tsum[0][:], tsum[0][:], tsum[1][:])

                def fin(final=False):
                    if final:
                        # last unit: process per 256-half so the dependent
                        # o_proj tiles start after ~half the chain latency
                        for hh in range(2):
                            hsl = slice(hh * 256, hh * 256 + 256)
                            osl = slice(tb * 512 + hh * 256,
                                        tb * 512 + hh * 256 + 256)
                            allred = prbf.tile([128, 512], F32, tag="rbf")
                            nc.gpsimd.partition_all_reduce(
                                allred[:, 0:256], tsum[0][:, hsl], 128,
                                bass_isa.ReduceOp.add)
                            rbf = prbf.tile([128, 512], F32, tag="rbf")
                            nc.vector.reciprocal(rbf[:, 0:256],
                                                 allred[:, 0:256])
                            nc.vector.tensor_mul(outT[l][:, osl],
                                                 psav[:, hsl],
                                                 rbf[:, 0:256])
                            xsplit(l, osl.start, osl.stop)
                        return
                    allred = prbf.tile([128, 512], F32, tag="rbf")
                    nc.gpsimd.partition_all_reduce(allred[:], tsum[0][:], 128,
                                                   bass_isa.ReduceOp.add)
                    rbf = prbf.tile([128, 512], F32, tag="rbf")
                    nc.vector.reciprocal(rbf[:], allred[:])
                    nc.vector.tensor_mul(outT[l][:, ts5], psav[:], rbf[:])
                    if tb == 3:
                        dnrow = prow.tile([1, 512], F32, tag="row512")
                        nc.vector.tensor_copy(dnrow[:], allred[0:1, :])
                        rrow = prow.tile([1, 512], F32, tag="row512")
                        nc.vector.tensor_copy(rrow[:], rbf[0:1, :])
                        state["cams"].append(
                            make_cam(l, E_cur[0], E_cur[1], erow16, vrow,
                                     dnrow, rrow))
                        # cols 1792:2048 split after the CaM add (cam_b)
                        xsplit(l, tb * 512, tb * 512 + 256)
                    else:
                        xsplit(l, tb * 512, tb * 512 + 512)
                state["fin"] = fin

            # ---- explicit emission sequence ----
            # two E sets are staged ahead of the AV pipeline (exp slack ~2
            # units); each av_unit piggybacks the scores of the unit TWO
            # ahead, so its qproj must already be emitted.
            qproj(0, 3)
            vproj(NB - 2)
            vproj(NB - 1)
            state["Eq"].append([scores_block(3, 0, kb) for kb in range(NB)])
            qproj(1, 0)
            state["Eq"].append([scores_block(3, 1, kb) for kb in range(NB)])
            av_unit(3, 0, (0, 0))
            qproj(2, 1)
            av_unit(3, 1, (0, 1))
            av_unit(0, 0, (1, 0))
            qproj(3, 2)
            av_unit(0, 1, (1, 1))
            for ti in (12, 13):
                oproj_ti(ti)
            av_unit(1, 0, (2, 0))
            for ti in (14, 15, 0, 1):
                oproj_ti(ti)
            av_unit(1, 1, (2, 1))
            for ti in (2, 3):
                oproj_ti(ti)
            av_unit(2, 0, None)
            # these overlap av(2,1), whose scores/exps already ran: the ACT
            # engine is idle there, so it can help with evacuations
            for ti in (4, 5):
                oproj_ti(ti, evac=("act", "dve"))
            av_unit(2, 1, None)
            # final fin first so its allred heads the Pool queue; ti 6,7
            # (tb1, ready) keep the PE fed while that chain runs, with
            # their evacs off Pool entirely
            state["fin"](final=True)
            state["fin"] = None
            # DVE owns the fin chain (recip, normalize, xsplit) that gates
            # oproj 8-11, so ti 6/7 evacuate via Pool; afterwards evacs
            # rotate three engines while SP+ACT carry only the piece-stores.
            oproj_ti(6, evac=("act", "dve"), split_store=True)
            oproj_ti(7, evac=("act", "dve"), split_store=True)
            for cam in state["cams"] + state["cam2"]:
                cam()
            for ti in (8, 9, 10, 11):
                oproj_ti(ti, evac=("dve", "act"), split_store=True)

    nc.compile()
    return nc


_NC_CACHE = None


def _get_nc():
    global _NC_CACHE
    if _NC_CACHE is None:
        _NC_CACHE = _build_nc()
    return _NC_CACHE


def _hilo(x):
    hi = x.astype(E4)
    lo = (x - hi.astype(np.float32)).astype(E4)
    return np.ascontiguousarray(hi), np.ascontiguousarray(lo)


def make_in_maps(hidden_states, Wq, Wk, Wv, Wo):
    hs = np.asarray(hidden_states, np.float32).reshape(T, DM)
    hs = np.nan_to_num(hs, nan=0.0, posinf=1e4, neginf=-1e4)
    hsT = np.ascontiguousarray(hs.T)                     # [dm, t]
    Wq = np.asarray(Wq, np.float32)
    Wk = np.asarray(Wk, np.float32)
    Wv = np.asarray(Wv, np.float32)
    Wo = np.asarray(Wo, np.float32)

    gidx = np.concatenate([np.arange(SB), np.arange(EV, T), [0]])

    # hst: [128, tb, kc, t] = hsT[kc*128+p, tb*512+t] * S_I
    A = (hsT * S_I).reshape(KC, 128, TB, 512)
    hst_hi, hst_lo = _hilo(A.transpose(1, 2, 0, 3))

    hskg = (hsT * S_I)[:, gidx].copy()
    hskg[:, NK - 1] = 0.0
    hsk_hi, hsk_lo = _hilo(hskg.reshape(KC, 128, NK).transpose(1, 0, 2))

    def pack_w(Wl):                                      # [2048, 256] ->
        return Wl.reshape(KC, 128, JC).transpose(1, 0, 2)  # [128, kc, j]

    inv_freq = 1.0 / (10000.0 ** (np.arange(0, D, 2, dtype=np.float32) / D))
    freqs = np.arange(T, dtype=np.float32)[:, None] * inv_freq[None, :]
    emb = np.concatenate([freqs, freqs], axis=-1)        # [T, D]
    cosT = np.cos(emb).T.astype(np.float32)              # [128, T]
    sinT = np.sin(emb).T.astype(np.float32)
    sinTs = np.concatenate([-sinT[:D // 2], sinT[D // 2:]], axis=0)
    trig = np.ascontiguousarray(np.concatenate(
        [cosT / PS_Q, sinTs / PS_Q, cosT[:, gidx] / PS_K,
         sinTs[:, gidx] / PS_K],
        axis=1)).astype(np.float16)

    import jax
    import jax.numpy as jnp
    u_full = np.asarray(
        jax.random.uniform(jax.random.key(42), (1, H), jnp.float32))

    scale = 1.0 / np.sqrt(np.float32(D))
    parts = np.arange(128)
    in_maps = []
    for c in range(NCORES):
        js = slice(c * JC, (c + 1) * JC)
        misc = np.zeros((128, 5), np.float32)
        misc[:, 0] = 1.0
        misc[:, 1] = (parts <= GEV_P).astype(np.float32)
        misc[:, 3] = u_full[0, c * HL]
        misc[:, 4] = u_full[0, c * HL + 1]
        wq_hi, wq_lo = _hilo(pack_w(Wq[js, :].T * (scale * S_WQ)))
        wk_hi, wk_lo = _hilo(pack_w(Wk[js, :].T * S_WK))
        wv_hi, wv_lo = _hilo(pack_w(Wv[js, :].T * S_WV))
        wo_hi, wo_lo = _hilo(
            (Wo[:, js].T * S_WO).reshape(HL, 128, DM).transpose(1, 0, 2))
        in_maps.append({
            "hst_hi": hst_hi, "hst_lo": hst_lo,
            "hsk_hi": hsk_hi, "hsk_lo": hsk_lo,
            "wq_hi": wq_hi, "wq_lo": wq_lo,
            "wk_hi": wk_hi, "wk_lo": wk_lo,
            "wv_hi": wv_hi, "wv_lo": wv_lo,
            "wo_hi": wo_hi, "wo_lo": wo_lo,
            "trig": trig,
            "misc": misc,
        })
    return in_maps


def kernel(hidden_states, Wq, Wk, Wv, Wo):
    nc = _get_nc()
    in_maps = make_in_maps(hidden_states, Wq, Wk, Wv, Wo)
    res = bass_utils.run_bass_kernel_spmd(nc, in_maps,
                                          core_ids=list(range(NCORES)))
    out = np.zeros((T, DM), np.float64)
    for c in range(NCORES):
        out += res.results[c]["po"].astype(np.float64)
    out /= PS_O
    out = np.nan_to_num(out.astype(np.float32), nan=0.0, posinf=1e4,
                        neginf=-1e4)
    return out.reshape(1, T, DM)
